# revision 7
# baseline (speedup 1.0000x reference)
"""Trainium2 kernel for nn_DeformableTransformer.

Strategy:
  - All large dense matmuls (value/offset/attw/output projections + FFN for the
    6 encoder layers, and the per-layer decoder value projections) run on the 8
    NeuronCores via Bass/Tile SPMD kernels, data-parallel over tokens
    (B*S = 16320 tokens -> 2040 per core, padded to 2048).
  - Matmul inputs are fp16 (fp32 accumulation in PSUM); everything is exact
    otherwise: layernorm / softmax / bilinear sampling / residuals are fp32.
  - Host performs the sharding, the deformable-attention bilinear gather
    (memory-bound scatter/gather glue) and the tiny decoder (Q=300) side ops.
"""

import sys
import numpy as np

sys.path.insert(0, "/opt/trn_rl_repo")

import concourse.bass as bass  # noqa: E402
from concourse import bacc  # noqa: E402
import concourse.mybir as mybir  # noqa: E402
from concourse.tile import TileContext  # noqa: E402
from concourse.bass_utils import run_bass_kernel_spmd  # noqa: E402

# ---------------------------------------------------------------- dims
D = 256
DFFN = 1024
NH = 8
DH = D // NH
L = 4
P = 4
NLE = 6
NLD = 6
B = 2
Q = 300
SHAPES = ((64, 96), (32, 48), (16, 24), (8, 12))
LSTART = [0, 6144, 7680, 8064]
S = 8160
NCORES = 8
TOK = (B * S) // NCORES      # 2040 real tokens per core
NTOK = 2048                  # padded per-core token count

_MM_CACHE = {}
DEV_CALLS = 0
DEV_WALL = 0.0


def _run_spmd(nc, in_maps):
    global DEV_CALLS, DEV_WALL
    import time as _t
    t0 = _t.time()
    res = run_bass_kernel_spmd(nc, in_maps, core_ids=list(range(NCORES)))
    DEV_WALL += _t.time() - t0
    DEV_CALLS += 1
    return res


def _build_mm(din, dout):
    """Bass kernel: out[dout, NTOK] (f32) = W[din, dout]^T @ xT[din, NTOK].

    xT and W are fp16 in DRAM; accumulation in PSUM f32.
    """
    nc = bacc.Bacc("TRN2", target_bir_lowering=False, debug=False,
                   num_devices=NCORES)
    xT = nc.dram_tensor("xT", [din, NTOK], mybir.dt.float16,
                        kind="ExternalInput")
    w = nc.dram_tensor("w", [din, dout], mybir.dt.float16,
                       kind="ExternalInput")
    out = nc.dram_tensor("out", [dout, NTOK], mybir.dt.float32,
                         kind="ExternalOutput")
    KT = din // 128          # k tiles
    MT = dout // 128         # output-partition tiles
    NT = NTOK // 512         # free-dim tiles
    with TileContext(nc) as tc:
        with (
            tc.tile_pool(name="wp", bufs=2) as wp,
            tc.tile_pool(name="xp", bufs=2) as xp,
            tc.tile_pool(name="pp", bufs=2, space="PSUM") as pp,
            tc.tile_pool(name="op", bufs=2) as op,
        ):
            # load full xT once: [128, KT*NTOK] fp16
            xt = xp.tile([128, KT * NTOK], mybir.dt.float16, tag="xt")
            xv = xt[:].rearrange("p (k n) -> k p n", k=KT)
            for k in range(KT):
                nc.sync.dma_start(xv[k], xT[k * 128:(k + 1) * 128, :])
            for m in range(MT):
                wt = wp.tile([128, KT * 128], mybir.dt.float16, tag="wt")
                wv = wt[:].rearrange("p (k c) -> k p c", k=KT)
                for k in range(KT):
                    nc.sync.dma_start(
                        wv[k], w[k * 128:(k + 1) * 128,
                                 m * 128:(m + 1) * 128])
                for n in range(NT):
                    ps = pp.tile([128, 512], mybir.dt.float32, tag="ps")
                    for k in range(KT):
                        nc.tensor.matmul(
                            ps[:], wv[k], xv[k][:, n * 512:(n + 1) * 512],
                            start=(k == 0), stop=(k == KT - 1))
                    ot = op.tile([128, 512], mybir.dt.float32, tag="ot")
                    nc.vector.tensor_copy(ot[:], ps[:])
                    nc.sync.dma_start(
                        out[m * 128:(m + 1) * 128,
                            n * 512:(n + 1) * 512], ot[:])
    nc.compile()
    return nc


def _get_mm(din, dout):
    key = (din, dout)
    if key not in _MM_CACHE:
        _MM_CACHE[key] = _build_mm(din, dout)
    return _MM_CACHE[key]


def _build_proj_bundle():
    """q = x + pos; off = off_w^T q; attw = attw_w^T q; val = val_w^T x."""
    nc = bacc.Bacc("TRN2", target_bir_lowering=False, debug=False,
                   num_devices=NCORES)
    xT = nc.dram_tensor("xT", [D, NTOK], mybir.dt.float16,
                        kind="ExternalInput")
    posT = nc.dram_tensor("posT", [D, NTOK], mybir.dt.float16,
                          kind="ExternalInput")
    w_off = nc.dram_tensor("w_off", [D, 256], mybir.dt.float16,
                           kind="ExternalInput")
    w_attw = nc.dram_tensor("w_attw", [D, 128], mybir.dt.float16,
                            kind="ExternalInput")
    w_val = nc.dram_tensor("w_val", [D, 256], mybir.dt.float16,
                           kind="ExternalInput")
    o_off = nc.dram_tensor("o_off", [256, NTOK], mybir.dt.float32,
                           kind="ExternalOutput")
    o_attw = nc.dram_tensor("o_attw", [128, NTOK], mybir.dt.float32,
                            kind="ExternalOutput")
    o_val = nc.dram_tensor("o_val", [256, NTOK], mybir.dt.float32,
                           kind="ExternalOutput")
    KT = 2
    NT = NTOK // 512
    with TileContext(nc) as tc:
        with (
            tc.tile_pool(name="wp", bufs=1) as wp,
            tc.tile_pool(name="xp", bufs=1) as xp,
            tc.tile_pool(name="pp", bufs=4, space="PSUM") as pp,
            tc.tile_pool(name="op", bufs=4) as op,
        ):
            xt = xp.tile([128, KT * NTOK], mybir.dt.float16, tag="xt")
            xv = xt[:].rearrange("p (k n) -> k p n", k=KT)
            pt = xp.tile([128, KT * NTOK], mybir.dt.float16, tag="pt")
            pv = pt[:].rearrange("p (k n) -> k p n", k=KT)
            qt = xp.tile([128, KT * NTOK], mybir.dt.float16, tag="qt")
            qv = qt[:].rearrange("p (k n) -> k p n", k=KT)
            for k in range(KT):
                nc.sync.dma_start(xv[k], xT[k * 128:(k + 1) * 128, :])
                nc.sync.dma_start(pv[k], posT[k * 128:(k + 1) * 128, :])
                nc.vector.tensor_add(qv[k], xv[k], pv[k])
            # weights: off (2 m-tiles), attw (1), val (2)
            allw = []
            for name, w, mt, rhs in (("off", w_off, 2, qv),
                                     ("attw", w_attw, 1, qv),
                                     ("val", w_val, 2, xv)):
                wt = wp.tile([128, KT * mt * 128], mybir.dt.float16,
                             tag=f"w{name}")
                wv = wt[:].rearrange("p (k m c) -> k m p c", k=KT, m=mt)
                for k in range(KT):
                    for m in range(mt):
                        nc.sync.dma_start(
                            wv[k, m], w[k * 128:(k + 1) * 128,
                                        m * 128:(m + 1) * 128])
                allw.append((wv, mt, rhs))
            for (wv, mt, rhs), o in zip(allw, (o_off, o_attw, o_val)):
                for m in range(mt):
                    for n in range(NT):
                        ps = pp.tile([128, 512], mybir.dt.float32, tag="ps")
                        for k in range(KT):
                            nc.tensor.matmul(
                                ps[:], wv[k, m],
                                rhs[k][:, n * 512:(n + 1) * 512],
                                start=(k == 0), stop=(k == KT - 1))
                        ot = op.tile([128, 512], mybir.dt.float32, tag="ot")
                        nc.vector.tensor_copy(ot[:], ps[:])
                        nc.sync.dma_start(
                            o[m * 128:(m + 1) * 128,
                              n * 512:(n + 1) * 512], ot[:])
    nc.compile()
    return nc


def _build_ffn():
    """out = w2^T relu(w1^T x + b1); host adds b2 and residual+LN."""
    nc = bacc.Bacc("TRN2", target_bir_lowering=False, debug=False,
                   num_devices=NCORES)
    xT = nc.dram_tensor("xT", [D, NTOK], mybir.dt.float16,
                        kind="ExternalInput")
    w1 = nc.dram_tensor("w1", [D, DFFN], mybir.dt.float16,
                        kind="ExternalInput")
    b1 = nc.dram_tensor("b1", [128, DFFN // 128], mybir.dt.float32,
                        kind="ExternalInput")
    w2 = nc.dram_tensor("w2", [DFFN, D], mybir.dt.float16,
                        kind="ExternalInput")
    out = nc.dram_tensor("out", [D, NTOK], mybir.dt.float32,
                         kind="ExternalOutput")
    KT = 2          # k tiles for first mm
    MT = DFFN // 128
    NT = NTOK // 512
    with TileContext(nc) as tc:
        with (
            tc.tile_pool(name="wp", bufs=1) as wp,
            tc.tile_pool(name="xp", bufs=1) as xp,
            tc.tile_pool(name="hp", bufs=1) as hp,
            tc.tile_pool(name="pp", bufs=4, space="PSUM") as pp,
            tc.tile_pool(name="op", bufs=4) as op,
        ):
            xt = xp.tile([128, KT * NTOK], mybir.dt.float16, tag="xt")
            xv = xt[:].rearrange("p (k n) -> k p n", k=KT)
            for k in range(KT):
                nc.sync.dma_start(xv[k], xT[k * 128:(k + 1) * 128, :])
            w1t = wp.tile([128, KT * MT * 128], mybir.dt.float16, tag="w1")
            w1v = w1t[:].rearrange("p (k m c) -> k m p c", k=KT, m=MT)
            b1t = wp.tile([128, MT], mybir.dt.float32, tag="b1")
            nc.sync.dma_start(b1t[:], b1[:])
            w2t = wp.tile([128, MT * 2 * 128], mybir.dt.float16, tag="w2")
            w2v = w2t[:].rearrange("p (k m c) -> k m p c", k=MT, m=2)
            for k in range(KT):
                for m in range(MT):
                    nc.sync.dma_start(
                        w1v[k, m], w1[k * 128:(k + 1) * 128,
                                      m * 128:(m + 1) * 128])
            for k in range(MT):
                for m in range(2):
                    nc.sync.dma_start(
                        w2v[k, m], w2[k * 128:(k + 1) * 128,
                                      m * 128:(m + 1) * 128])
            ht = hp.tile([128, MT * NTOK], mybir.dt.float16, tag="ht")
            hv = ht[:].rearrange("p (m n) -> m p n", m=MT)
            for m in range(MT):
                for n in range(NT):
                    ps = pp.tile([128, 512], mybir.dt.float32, tag="ps")
                    for k in range(KT):
                        nc.tensor.matmul(
                            ps[:], w1v[k, m], xv[k][:, n * 512:(n + 1) * 512],
                            start=(k == 0), stop=(k == KT - 1))
                    # relu(ps + b1) -> fp16 h
                    nc.scalar.activation(
                        hv[m][:, n * 512:(n + 1) * 512], ps[:],
                        mybir.ActivationFunctionType.Relu,
                        bias=b1t[:, m:m + 1], scale=1.0)
            for m in range(2):
                for n in range(NT):
                    ps = pp.tile([128, 512], mybir.dt.float32, tag="ps2")
                    for k in range(MT):
                        nc.tensor.matmul(
                            ps[:], w2v[k, m], hv[k][:, n * 512:(n + 1) * 512],
                            start=(k == 0), stop=(k == MT - 1))
                    ot = op.tile([128, 512], mybir.dt.float32, tag="ot")
                    nc.vector.tensor_copy(ot[:], ps[:])
                    nc.sync.dma_start(
                        out[m * 128:(m + 1) * 128,
                            n * 512:(n + 1) * 512], ot[:])
    nc.compile()
    return nc


def _to_shards_T(x2d, din):
    xs = x2d.reshape(NCORES, TOK, din)
    outs = []
    for c in range(NCORES):
        xt = np.zeros((din, NTOK), np.float16)
        xt[:, :TOK] = xs[c].T.astype(np.float16)
        outs.append(xt)
    return outs


def _from_shards_T(res, name, dout):
    return np.concatenate([r[name][:dout, :TOK].T for r in res.results], 0)


def _dev_proj_bundle(x, pos, p):
    """Returns (off[N,256], attw[N,128], val[N,256]) for N=B*S tokens."""
    if "proj" not in _MM_CACHE:
        _MM_CACHE["proj"] = _build_proj_bundle()
    nc = _MM_CACHE["proj"]
    xsh = _to_shards_T(x, D)
    psh = _to_shards_T(pos, D)
    w_off = p["off_w"].astype(np.float16)
    w_attw = p["attw_w"].astype(np.float16)
    w_val = p["val_w"].astype(np.float16)
    in_maps = [{"xT": xsh[c], "posT": psh[c], "w_off": w_off,
                "w_attw": w_attw, "w_val": w_val} for c in range(NCORES)]
    res = _run_spmd(nc, in_maps)
    return (_from_shards_T(res, "o_off", 256),
            _from_shards_T(res, "o_attw", 128),
            _from_shards_T(res, "o_val", 256))


def _dev_ffn(x, p):
    if "ffn" not in _MM_CACHE:
        _MM_CACHE["ffn"] = _build_ffn()
    nc = _MM_CACHE["ffn"]
    xsh = _to_shards_T(x, D)
    w1 = p["f1_w"].astype(np.float16)
    b1 = np.ascontiguousarray(
        p["f1_b"].astype(np.float32).reshape(DFFN // 128, 128).T)
    w2 = p["f2_w"].astype(np.float16)
    in_maps = [{"xT": xsh[c], "w1": w1, "b1": b1, "w2": w2}
               for c in range(NCORES)]
    res = _run_spmd(nc, in_maps)
    return _from_shards_T(res, "out", 256)


def _dev_mm(x, w):
    """x: [N, din] f32 (N = NCORES*TOK tokens), w: [din, dout] f32.
    Returns x @ w computed on the 8 NeuronCores (fp16 matmul, f32 accum)."""
    n, din = x.shape
    dout_real = w.shape[1]
    dout = max(128, int(np.ceil(dout_real / 128)) * 128)
    if dout % 256:
        dout += 128
    # pad dout so kernel dout is multiple of 256 (keeps kernel count small)
    for cand in (256, 768, 1024):
        if dout_real <= cand:
            dout = cand
            break
    nc = _get_mm(din, dout)
    wp = np.zeros((din, dout), np.float16)
    wp[:, :dout_real] = w.astype(np.float16)
    assert n == NCORES * TOK, (n, NCORES * TOK)
    xs = x.reshape(NCORES, TOK, din)
    in_maps = []
    for c in range(NCORES):
        xt = np.zeros((din, NTOK), np.float16)
        xt[:, :TOK] = xs[c].T.astype(np.float16)
        in_maps.append({"xT": xt, "w": wp})
    res = _run_spmd(nc, in_maps)
    outs = [r["out"][:dout_real, :TOK].T for r in res.results]
    return np.concatenate(outs, 0)


# ---------------------------------------------------------------- host math
def _layer_norm(x, g, b):
    m = x.mean(-1, keepdims=True)
    v = x.var(-1, keepdims=True)
    return (x - m) / np.sqrt(v + 1e-5) * g + b


def _softmax(x, axis):
    x = x - x.max(axis, keepdims=True)
    e = np.exp(x)
    return e / e.sum(axis, keepdims=True)


def _enc_reference_points():
    pts = []
    for (H_, W_) in SHAPES:
        ry, rx = np.meshgrid(
            np.linspace(0.5, H_ - 0.5, H_) / H_,
            np.linspace(0.5, W_ - 0.5, W_) / W_, indexing="ij")
        pts.append(np.stack([rx.reshape(-1), ry.reshape(-1)], -1))
    g = np.concatenate(pts, 0).astype(np.float32)
    return np.broadcast_to(g[None, :, None, :], (B, S, L, 2))


def _sample(value, loc, attw):
    """value: (B,NH,S,DH) f32; loc: (B,NH,Qn,L,P,2); attw: (B,NH,Qn,L,P).
    Returns (B,NH,Qn,DH)."""
    Bn, _, Qn = loc.shape[:3]
    out = np.zeros((Bn, NH, Qn, DH), np.float32)
    for l, (Hl, Wl) in enumerate(SHAPES):
        v = value[:, :, LSTART[l]:LSTART[l] + Hl * Wl]   # (B,NH,HW,DH)
        x = loc[:, :, :, l, :, 0] * Wl - 0.5             # (B,NH,Qn,P)
        y = loc[:, :, :, l, :, 1] * Hl - 0.5
        x0 = np.floor(x)
        y0 = np.floor(y)
        samp = np.zeros((Bn, NH, Qn, P, DH), np.float32)
        vflat = v.reshape(Bn * NH, Hl * Wl, DH)
        for dy in (0.0, 1.0):
            for dx in (0.0, 1.0):
                xi = x0 + dx
                yi = y0 + dy
                w_ = (1.0 - np.abs(x - xi)) * (1.0 - np.abs(y - yi))
                valid = (xi >= 0) & (xi < Wl) & (yi >= 0) & (yi < Hl)
                idx = (np.clip(yi, 0, Hl - 1) * Wl
                       + np.clip(xi, 0, Wl - 1)).astype(np.int64)
                idx = idx.reshape(Bn * NH, Qn * P)
                gv = np.take_along_axis(vflat, idx[:, :, None], axis=1)
                gv = gv.reshape(Bn, NH, Qn, P, DH)
                samp += gv * (w_ * valid)[..., None]
        out += (samp * attw[:, :, :, l, :, None]).sum(3)
    return out


def _msda(query, ref, value, p, dev_out_proj=True):
    """query: (B,Qn,D); ref: (B,Qn,L,2); value: (B,NH,S,DH) already projected."""
    Bn, Qn, _ = query.shape
    qf = query.reshape(Bn * Qn, D)
    if Qn == S:
        off = _dev_mm(qf, p["off_w"]) + p["off_b"]
        aw = _dev_mm(qf, p["attw_w"]) + p["attw_b"]
    else:
        off = qf @ p["off_w"] + p["off_b"]
        aw = qf @ p["attw_w"] + p["attw_b"]
    off = off.reshape(Bn, Qn, NH, L, P, 2)
    attw = _softmax(aw.reshape(Bn, Qn, NH, L * P), -1)
    attw = attw.reshape(Bn, Qn, NH, L, P).transpose(0, 2, 1, 3, 4)
    norm = np.array([[w, h] for (h, w) in SHAPES], np.float32)
    loc = ref[:, :, None, :, None, :] + off / norm[None, None, None, :, None, :]
    loc = loc.transpose(0, 2, 1, 3, 4, 5)
    out = _sample(value, loc, attw)                      # (B,NH,Qn,DH)
    out = out.transpose(0, 2, 1, 3).reshape(Bn * Qn, D)
    if dev_out_proj and Qn == S:
        return (_dev_mm(out, p["out_w"]) + p["out_b"]).reshape(Bn, Qn, D)
    return (out @ p["out_w"] + p["out_b"]).reshape(Bn, Qn, D)


def _project_value(src, p):
    vf = _dev_mm(src.reshape(B * S, D), p["val_w"]) + p["val_b"]
    return vf.reshape(B, S, NH, DH).transpose(0, 2, 1, 3)


def kernel(src_flatten, pos_embed, query_embed, ref_w, ref_b,
           enc_params, dec_params):
    src_flatten = np.asarray(src_flatten, np.float32)
    pos_embed = np.asarray(pos_embed, np.float32)
    query_embed = np.asarray(query_embed, np.float32)
    ref_w = np.asarray(ref_w, np.float32)
    ref_b = np.asarray(ref_b, np.float32)
    enc_params = {k: np.asarray(v, np.float32) for k, v in enc_params.items()}
    dec_params = {k: np.asarray(v, np.float32) for k, v in dec_params.items()}

    ref_enc = _enc_reference_points()

    # ------------------------------------------------ encoder
    norm = np.array([[w, h] for (h, w) in SHAPES], np.float32)
    x = src_flatten
    for li in range(NLE):
        p = {k: v[li] for k, v in enc_params.items()}
        off, aw, vf = _dev_proj_bundle(
            x.reshape(B * S, D), pos_embed.reshape(B * S, D), p)
        value = (vf + p["val_b"]).reshape(B, S, NH, DH).transpose(0, 2, 1, 3)
        off = (off + p["off_b"]).reshape(B, S, NH, L, P, 2)
        aw = aw + p["attw_b"]
        attw = _softmax(aw.reshape(B, S, NH, L * P), -1)
        attw = attw.reshape(B, S, NH, L, P).transpose(0, 2, 1, 3, 4)
        loc = (ref_enc[:, :, None, :, None, :]
               + off / norm[None, None, None, :, None, :])
        loc = loc.transpose(0, 2, 1, 3, 4, 5)
        sam = _sample(value, loc, attw)
        sam = sam.transpose(0, 2, 1, 3).reshape(B * S, D)
        attn = (_dev_mm(sam, p["out_w"]) + p["out_b"]).reshape(B, S, D)
        x = _layer_norm(x + attn, p["ln1_g"], p["ln1_b"])
        ff = _dev_ffn(x.reshape(B * S, D), p) + p["f2_b"]
        x = _layer_norm(x + ff.reshape(B, S, D), p["ln2_g"], p["ln2_b"])
    memory = x

    # ------------------------------------------------ decoder
    qpos = np.broadcast_to(query_embed[None, :, :D], (B, Q, D))
    tgt = np.broadcast_to(query_embed[None, :, D:], (B, Q, D)).copy()
    ref = 1.0 / (1.0 + np.exp(-(query_embed[:, :D] @ ref_w + ref_b)))
    ref_in = np.broadcast_to(ref[None, :, None, :], (B, Q, L, 2))
    scale = 1.0 / np.sqrt(DH)

    x = tgt
    for li in range(NLD):
        p = {k: v[li] for k, v in dec_params.items()}
        q = x + qpos
        qh = (q @ p["qkv_w"][:, :D] + p["qkv_b"][:D]).reshape(B, Q, NH, DH)
        kh = (q @ p["qkv_w"][:, D:2 * D]
              + p["qkv_b"][D:2 * D]).reshape(B, Q, NH, DH)
        vh = (x @ p["qkv_w"][:, 2 * D:]
              + p["qkv_b"][2 * D:]).reshape(B, Q, NH, DH)
        att = _softmax(
            np.einsum("bqhd,bkhd->bhqk", qh, kh) * scale, -1)
        sa = (np.einsum("bhqk,bkhd->bqhd", att, vh).reshape(B, Q, D)
              @ p["oa_w"] + p["oa_b"])
        x = _layer_norm(x + sa, p["ln2_g"], p["ln2_b"])
        value = _project_value(memory, p)
        attn = _msda(x + qpos, ref_in, value, p)
        x = _layer_norm(x + attn, p["ln1_g"], p["ln1_b"])
        h = np.maximum(x @ p["f1_w"] + p["f1_b"], 0.0)
        x = _layer_norm(x + h @ p["f2_w"] + p["f2_b"],
                        p["ln3_g"], p["ln3_b"])
    return x.astype(np.float32)


# revision 8
# speedup vs baseline: 1.1916x; 1.1916x over previous
"""Trainium2 kernel for nn_DeformableTransformer.

Strategy:
  - All large dense matmuls (value/offset/attw/output projections + FFN for the
    6 encoder layers, and the per-layer decoder value projections) run on the 8
    NeuronCores via Bass/Tile SPMD kernels, data-parallel over tokens
    (B*S = 16320 tokens -> 2040 per core, padded to 2048).
  - Matmul inputs are fp16 (fp32 accumulation in PSUM); everything is exact
    otherwise: layernorm / softmax / bilinear sampling / residuals are fp32.
  - Host performs the sharding, the deformable-attention bilinear gather
    (memory-bound scatter/gather glue) and the tiny decoder (Q=300) side ops.
"""

import sys
import numpy as np

sys.path.insert(0, "/opt/trn_rl_repo")

import concourse.bass as bass  # noqa: E402
from concourse import bacc  # noqa: E402
import concourse.mybir as mybir  # noqa: E402
from concourse.tile import TileContext  # noqa: E402
from concourse.bass_utils import run_bass_kernel_spmd  # noqa: E402

# ---------------------------------------------------------------- dims
D = 256
DFFN = 1024
NH = 8
DH = D // NH
L = 4
P = 4
NLE = 6
NLD = 6
B = 2
Q = 300
SHAPES = ((64, 96), (32, 48), (16, 24), (8, 12))
LSTART = [0, 6144, 7680, 8064]
S = 8160
NCORES = 8
TOK = (B * S) // NCORES      # 2040 real tokens per core
NTOK = 2048                  # padded per-core token count

_MM_CACHE = {}
DEV_CALLS = 0
DEV_WALL = 0.0


_JIT_CACHE = {}


def _get_spmd_fn(nc):
    """Memoized PJRT executable for a compiled Bass module.

    run_bass_kernel_spmd rebuilds its jit closure per call (cache miss and
    full retrace each time); this builds the same shard_map program once and
    reuses it, which removes ~1s of dispatch overhead per device call.
    """
    key = id(nc)
    if key in _JIT_CACHE:
        return _JIT_CACHE[key]
    import jax
    from jax.experimental.shard_map import shard_map
    from jax.sharding import Mesh, PartitionSpec
    from concourse import bass2jax
    bass2jax.install_neuronx_cc_hook()
    partition_name = (nc.partition_id_tensor.name
                      if nc.partition_id_tensor else None)
    in_names, out_names, out_avals, zero_outs = [], [], [], []
    for alloc in nc.m.functions[0].allocations:
        if not isinstance(alloc, mybir.MemoryLocationSet):
            continue
        name = alloc.memorylocations[0].name
        if alloc.kind == "ExternalInput":
            if name != partition_name:
                in_names.append(name)
        elif alloc.kind == "ExternalOutput":
            shape = tuple(alloc.tensor_shape)
            dtype = mybir.dt.np(alloc.dtype)
            out_names.append(name)
            out_avals.append(jax.core.ShapedArray(shape, dtype))
            zero_outs.append(np.zeros(shape, dtype))
    n_params = len(in_names)
    all_names = list(in_names) + list(out_names)
    if partition_name is not None:
        all_names.append(partition_name)
    donate = tuple(range(n_params, n_params + len(out_names)))

    def _body(*args):
        operands = list(args)
        if partition_name is not None:
            operands.append(bass2jax.partition_id_tensor())
        return tuple(bass2jax._bass_exec_p.bind(
            *operands,
            out_avals=tuple(out_avals),
            in_names=tuple(all_names),
            out_names=tuple(out_names),
            lowering_input_output_aliases=(),
            sim_require_finite=True,
            sim_require_nnan=True,
            nc=nc,
        ))

    devices = jax.devices()[:NCORES]
    mesh = Mesh(np.asarray(devices), ("core",))
    nin = n_params + len(out_names)
    sharded = jax.jit(
        shard_map(_body, mesh=mesh,
                  in_specs=(PartitionSpec("core"),) * nin,
                  out_specs=(PartitionSpec("core"),) * len(out_names),
                  check_rep=False),
        donate_argnums=donate, keep_unused=True)

    def run(in_maps):
        per_core = [[np.asarray(m[nm]) for nm in in_names] for m in in_maps]
        concat_in = [
            np.concatenate([per_core[c][i] for c in range(NCORES)], axis=0)
            for i in range(n_params)
        ]
        concat_zero = [
            np.concatenate([z] * NCORES, axis=0) for z in zero_outs
        ]
        outs = sharded(*concat_in, *concat_zero)
        results = []
        for c in range(NCORES):
            res = {}
            for i, nm in enumerate(out_names):
                arr = np.asarray(outs[i])
                per = arr.shape[0] // NCORES
                res[nm] = arr[c * per:(c + 1) * per]
            results.append(res)
        return results

    _JIT_CACHE[key] = run
    return run


def _run_spmd(nc, in_maps):
    global DEV_CALLS, DEV_WALL
    import time as _t

    class _R:
        pass

    t0 = _t.time()
    try:
        results = _get_spmd_fn(nc)(in_maps)
        r = _R()
        r.results = results
    except Exception:
        _JIT_CACHE[id(nc)] = None
        _JIT_CACHE.pop(id(nc), None)
        r = run_bass_kernel_spmd(nc, in_maps, core_ids=list(range(NCORES)))
    DEV_WALL += _t.time() - t0
    DEV_CALLS += 1
    return r


def _build_mm(din, dout):
    """Bass kernel: out[dout, NTOK] (f32) = W[din, dout]^T @ xT[din, NTOK].

    xT and W are fp16 in DRAM; accumulation in PSUM f32.
    """
    nc = bacc.Bacc("TRN2", target_bir_lowering=False, debug=False,
                   num_devices=NCORES)
    xT = nc.dram_tensor("xT", [din, NTOK], mybir.dt.float16,
                        kind="ExternalInput")
    w = nc.dram_tensor("w", [din, dout], mybir.dt.float16,
                       kind="ExternalInput")
    out = nc.dram_tensor("out", [dout, NTOK], mybir.dt.float32,
                         kind="ExternalOutput")
    KT = din // 128          # k tiles
    MT = dout // 128         # output-partition tiles
    NT = NTOK // 512         # free-dim tiles
    with TileContext(nc) as tc:
        with (
            tc.tile_pool(name="wp", bufs=2) as wp,
            tc.tile_pool(name="xp", bufs=2) as xp,
            tc.tile_pool(name="pp", bufs=2, space="PSUM") as pp,
            tc.tile_pool(name="op", bufs=2) as op,
        ):
            # load full xT once: [128, KT*NTOK] fp16
            xt = xp.tile([128, KT * NTOK], mybir.dt.float16, tag="xt")
            xv = xt[:].rearrange("p (k n) -> k p n", k=KT)
            for k in range(KT):
                nc.sync.dma_start(xv[k], xT[k * 128:(k + 1) * 128, :])
            for m in range(MT):
                wt = wp.tile([128, KT * 128], mybir.dt.float16, tag="wt")
                wv = wt[:].rearrange("p (k c) -> k p c", k=KT)
                for k in range(KT):
                    nc.sync.dma_start(
                        wv[k], w[k * 128:(k + 1) * 128,
                                 m * 128:(m + 1) * 128])
                for n in range(NT):
                    ps = pp.tile([128, 512], mybir.dt.float32, tag="ps")
                    for k in range(KT):
                        nc.tensor.matmul(
                            ps[:], wv[k], xv[k][:, n * 512:(n + 1) * 512],
                            start=(k == 0), stop=(k == KT - 1))
                    ot = op.tile([128, 512], mybir.dt.float32, tag="ot")
                    nc.vector.tensor_copy(ot[:], ps[:])
                    nc.sync.dma_start(
                        out[m * 128:(m + 1) * 128,
                            n * 512:(n + 1) * 512], ot[:])
    nc.compile()
    return nc


def _get_mm(din, dout):
    key = (din, dout)
    if key not in _MM_CACHE:
        _MM_CACHE[key] = _build_mm(din, dout)
    return _MM_CACHE[key]


def _build_proj_bundle():
    """q = x + pos; off = off_w^T q; attw = attw_w^T q; val = val_w^T x."""
    nc = bacc.Bacc("TRN2", target_bir_lowering=False, debug=False,
                   num_devices=NCORES)
    xT = nc.dram_tensor("xT", [D, NTOK], mybir.dt.float16,
                        kind="ExternalInput")
    posT = nc.dram_tensor("posT", [D, NTOK], mybir.dt.float16,
                          kind="ExternalInput")
    w_off = nc.dram_tensor("w_off", [D, 256], mybir.dt.float16,
                           kind="ExternalInput")
    w_attw = nc.dram_tensor("w_attw", [D, 128], mybir.dt.float16,
                            kind="ExternalInput")
    w_val = nc.dram_tensor("w_val", [D, 256], mybir.dt.float16,
                           kind="ExternalInput")
    o_off = nc.dram_tensor("o_off", [256, NTOK], mybir.dt.float32,
                           kind="ExternalOutput")
    o_attw = nc.dram_tensor("o_attw", [128, NTOK], mybir.dt.float32,
                            kind="ExternalOutput")
    o_val = nc.dram_tensor("o_val", [256, NTOK], mybir.dt.float32,
                           kind="ExternalOutput")
    KT = 2
    NT = NTOK // 512
    with TileContext(nc) as tc:
        with (
            tc.tile_pool(name="wp", bufs=1) as wp,
            tc.tile_pool(name="xp", bufs=1) as xp,
            tc.tile_pool(name="pp", bufs=4, space="PSUM") as pp,
            tc.tile_pool(name="op", bufs=4) as op,
        ):
            xt = xp.tile([128, KT * NTOK], mybir.dt.float16, tag="xt")
            xv = xt[:].rearrange("p (k n) -> k p n", k=KT)
            pt = xp.tile([128, KT * NTOK], mybir.dt.float16, tag="pt")
            pv = pt[:].rearrange("p (k n) -> k p n", k=KT)
            qt = xp.tile([128, KT * NTOK], mybir.dt.float16, tag="qt")
            qv = qt[:].rearrange("p (k n) -> k p n", k=KT)
            for k in range(KT):
                nc.sync.dma_start(xv[k], xT[k * 128:(k + 1) * 128, :])
                nc.sync.dma_start(pv[k], posT[k * 128:(k + 1) * 128, :])
                nc.vector.tensor_add(qv[k], xv[k], pv[k])
            # weights: off (2 m-tiles), attw (1), val (2)
            allw = []
            for name, w, mt, rhs in (("off", w_off, 2, qv),
                                     ("attw", w_attw, 1, qv),
                                     ("val", w_val, 2, xv)):
                wt = wp.tile([128, KT * mt * 128], mybir.dt.float16,
                             tag=f"w{name}")
                wv = wt[:].rearrange("p (k m c) -> k m p c", k=KT, m=mt)
                for k in range(KT):
                    for m in range(mt):
                        nc.sync.dma_start(
                            wv[k, m], w[k * 128:(k + 1) * 128,
                                        m * 128:(m + 1) * 128])
                allw.append((wv, mt, rhs))
            for (wv, mt, rhs), o in zip(allw, (o_off, o_attw, o_val)):
                for m in range(mt):
                    for n in range(NT):
                        ps = pp.tile([128, 512], mybir.dt.float32, tag="ps")
                        for k in range(KT):
                            nc.tensor.matmul(
                                ps[:], wv[k, m],
                                rhs[k][:, n * 512:(n + 1) * 512],
                                start=(k == 0), stop=(k == KT - 1))
                        ot = op.tile([128, 512], mybir.dt.float32, tag="ot")
                        nc.vector.tensor_copy(ot[:], ps[:])
                        nc.sync.dma_start(
                            o[m * 128:(m + 1) * 128,
                              n * 512:(n + 1) * 512], ot[:])
    nc.compile()
    return nc


def _build_ffn():
    """out = w2^T relu(w1^T x + b1); host adds b2 and residual+LN."""
    nc = bacc.Bacc("TRN2", target_bir_lowering=False, debug=False,
                   num_devices=NCORES)
    xT = nc.dram_tensor("xT", [D, NTOK], mybir.dt.float16,
                        kind="ExternalInput")
    w1 = nc.dram_tensor("w1", [D, DFFN], mybir.dt.float16,
                        kind="ExternalInput")
    b1 = nc.dram_tensor("b1", [128, DFFN // 128], mybir.dt.float32,
                        kind="ExternalInput")
    w2 = nc.dram_tensor("w2", [DFFN, D], mybir.dt.float16,
                        kind="ExternalInput")
    out = nc.dram_tensor("out", [D, NTOK], mybir.dt.float32,
                         kind="ExternalOutput")
    KT = 2          # k tiles for first mm
    MT = DFFN // 128
    NT = NTOK // 512
    with TileContext(nc) as tc:
        with (
            tc.tile_pool(name="wp", bufs=1) as wp,
            tc.tile_pool(name="xp", bufs=1) as xp,
            tc.tile_pool(name="hp", bufs=1) as hp,
            tc.tile_pool(name="pp", bufs=4, space="PSUM") as pp,
            tc.tile_pool(name="op", bufs=4) as op,
        ):
            xt = xp.tile([128, KT * NTOK], mybir.dt.float16, tag="xt")
            xv = xt[:].rearrange("p (k n) -> k p n", k=KT)
            for k in range(KT):
                nc.sync.dma_start(xv[k], xT[k * 128:(k + 1) * 128, :])
            w1t = wp.tile([128, KT * MT * 128], mybir.dt.float16, tag="w1")
            w1v = w1t[:].rearrange("p (k m c) -> k m p c", k=KT, m=MT)
            b1t = wp.tile([128, MT], mybir.dt.float32, tag="b1")
            nc.sync.dma_start(b1t[:], b1[:])
            w2t = wp.tile([128, MT * 2 * 128], mybir.dt.float16, tag="w2")
            w2v = w2t[:].rearrange("p (k m c) -> k m p c", k=MT, m=2)
            for k in range(KT):
                for m in range(MT):
                    nc.sync.dma_start(
                        w1v[k, m], w1[k * 128:(k + 1) * 128,
                                      m * 128:(m + 1) * 128])
            for k in range(MT):
                for m in range(2):
                    nc.sync.dma_start(
                        w2v[k, m], w2[k * 128:(k + 1) * 128,
                                      m * 128:(m + 1) * 128])
            ht = hp.tile([128, MT * NTOK], mybir.dt.float16, tag="ht")
            hv = ht[:].rearrange("p (m n) -> m p n", m=MT)
            for m in range(MT):
                for n in range(NT):
                    ps = pp.tile([128, 512], mybir.dt.float32, tag="ps")
                    for k in range(KT):
                        nc.tensor.matmul(
                            ps[:], w1v[k, m], xv[k][:, n * 512:(n + 1) * 512],
                            start=(k == 0), stop=(k == KT - 1))
                    # relu(ps + b1) -> fp16 h
                    nc.scalar.activation(
                        hv[m][:, n * 512:(n + 1) * 512], ps[:],
                        mybir.ActivationFunctionType.Relu,
                        bias=b1t[:, m:m + 1], scale=1.0)
            for m in range(2):
                for n in range(NT):
                    ps = pp.tile([128, 512], mybir.dt.float32, tag="ps2")
                    for k in range(MT):
                        nc.tensor.matmul(
                            ps[:], w2v[k, m], hv[k][:, n * 512:(n + 1) * 512],
                            start=(k == 0), stop=(k == MT - 1))
                    ot = op.tile([128, 512], mybir.dt.float32, tag="ot")
                    nc.vector.tensor_copy(ot[:], ps[:])
                    nc.sync.dma_start(
                        out[m * 128:(m + 1) * 128,
                            n * 512:(n + 1) * 512], ot[:])
    nc.compile()
    return nc


def _to_shards_T(x2d, din):
    xs = x2d.reshape(NCORES, TOK, din)
    outs = []
    for c in range(NCORES):
        xt = np.zeros((din, NTOK), np.float16)
        xt[:, :TOK] = xs[c].T.astype(np.float16)
        outs.append(xt)
    return outs


def _from_shards_T(res, name, dout):
    return np.concatenate([r[name][:dout, :TOK].T for r in res.results], 0)


def _dev_proj_bundle(x, pos, p):
    """Returns (off[N,256], attw[N,128], val[N,256]) for N=B*S tokens."""
    if "proj" not in _MM_CACHE:
        _MM_CACHE["proj"] = _build_proj_bundle()
    nc = _MM_CACHE["proj"]
    xsh = _to_shards_T(x, D)
    psh = _to_shards_T(pos, D)
    w_off = p["off_w"].astype(np.float16)
    w_attw = p["attw_w"].astype(np.float16)
    w_val = p["val_w"].astype(np.float16)
    in_maps = [{"xT": xsh[c], "posT": psh[c], "w_off": w_off,
                "w_attw": w_attw, "w_val": w_val} for c in range(NCORES)]
    res = _run_spmd(nc, in_maps)
    return (_from_shards_T(res, "o_off", 256),
            _from_shards_T(res, "o_attw", 128),
            _from_shards_T(res, "o_val", 256))


def _dev_ffn(x, p):
    if "ffn" not in _MM_CACHE:
        _MM_CACHE["ffn"] = _build_ffn()
    nc = _MM_CACHE["ffn"]
    xsh = _to_shards_T(x, D)
    w1 = p["f1_w"].astype(np.float16)
    b1 = np.ascontiguousarray(
        p["f1_b"].astype(np.float32).reshape(DFFN // 128, 128).T)
    w2 = p["f2_w"].astype(np.float16)
    in_maps = [{"xT": xsh[c], "w1": w1, "b1": b1, "w2": w2}
               for c in range(NCORES)]
    res = _run_spmd(nc, in_maps)
    return _from_shards_T(res, "out", 256)


def _dev_mm(x, w):
    """x: [N, din] f32 (N = NCORES*TOK tokens), w: [din, dout] f32.
    Returns x @ w computed on the 8 NeuronCores (fp16 matmul, f32 accum)."""
    n, din = x.shape
    dout_real = w.shape[1]
    dout = max(128, int(np.ceil(dout_real / 128)) * 128)
    if dout % 256:
        dout += 128
    # pad dout so kernel dout is multiple of 256 (keeps kernel count small)
    for cand in (256, 768, 1024):
        if dout_real <= cand:
            dout = cand
            break
    nc = _get_mm(din, dout)
    wp = np.zeros((din, dout), np.float16)
    wp[:, :dout_real] = w.astype(np.float16)
    assert n == NCORES * TOK, (n, NCORES * TOK)
    xs = x.reshape(NCORES, TOK, din)
    in_maps = []
    for c in range(NCORES):
        xt = np.zeros((din, NTOK), np.float16)
        xt[:, :TOK] = xs[c].T.astype(np.float16)
        in_maps.append({"xT": xt, "w": wp})
    res = _run_spmd(nc, in_maps)
    outs = [r["out"][:dout_real, :TOK].T for r in res.results]
    return np.concatenate(outs, 0)


# ---------------------------------------------------------------- host math
def _layer_norm(x, g, b):
    m = x.mean(-1, keepdims=True)
    v = x.var(-1, keepdims=True)
    return (x - m) / np.sqrt(v + 1e-5) * g + b


def _softmax(x, axis):
    x = x - x.max(axis, keepdims=True)
    e = np.exp(x)
    return e / e.sum(axis, keepdims=True)


def _enc_reference_points():
    pts = []
    for (H_, W_) in SHAPES:
        ry, rx = np.meshgrid(
            np.linspace(0.5, H_ - 0.5, H_) / H_,
            np.linspace(0.5, W_ - 0.5, W_) / W_, indexing="ij")
        pts.append(np.stack([rx.reshape(-1), ry.reshape(-1)], -1))
    g = np.concatenate(pts, 0).astype(np.float32)
    return np.broadcast_to(g[None, :, None, :], (B, S, L, 2))


def _sample(value, loc, attw):
    """value: (B,NH,S,DH) f32; loc: (B,NH,Qn,L,P,2); attw: (B,NH,Qn,L,P).
    Returns (B,NH,Qn,DH)."""
    Bn, _, Qn = loc.shape[:3]
    out = np.zeros((Bn, NH, Qn, DH), np.float32)
    for l, (Hl, Wl) in enumerate(SHAPES):
        v = value[:, :, LSTART[l]:LSTART[l] + Hl * Wl]   # (B,NH,HW,DH)
        x = loc[:, :, :, l, :, 0] * Wl - 0.5             # (B,NH,Qn,P)
        y = loc[:, :, :, l, :, 1] * Hl - 0.5
        x0 = np.floor(x)
        y0 = np.floor(y)
        samp = np.zeros((Bn, NH, Qn, P, DH), np.float32)
        vflat = v.reshape(Bn * NH, Hl * Wl, DH)
        for dy in (0.0, 1.0):
            for dx in (0.0, 1.0):
                xi = x0 + dx
                yi = y0 + dy
                w_ = (1.0 - np.abs(x - xi)) * (1.0 - np.abs(y - yi))
                valid = (xi >= 0) & (xi < Wl) & (yi >= 0) & (yi < Hl)
                idx = (np.clip(yi, 0, Hl - 1) * Wl
                       + np.clip(xi, 0, Wl - 1)).astype(np.int64)
                idx = idx.reshape(Bn * NH, Qn * P)
                gv = np.take_along_axis(vflat, idx[:, :, None], axis=1)
                gv = gv.reshape(Bn, NH, Qn, P, DH)
                samp += gv * (w_ * valid)[..., None]
        out += (samp * attw[:, :, :, l, :, None]).sum(3)
    return out


def _msda(query, ref, value, p, dev_out_proj=True):
    """query: (B,Qn,D); ref: (B,Qn,L,2); value: (B,NH,S,DH) already projected."""
    Bn, Qn, _ = query.shape
    qf = query.reshape(Bn * Qn, D)
    if Qn == S:
        off = _dev_mm(qf, p["off_w"]) + p["off_b"]
        aw = _dev_mm(qf, p["attw_w"]) + p["attw_b"]
    else:
        off = qf @ p["off_w"] + p["off_b"]
        aw = qf @ p["attw_w"] + p["attw_b"]
    off = off.reshape(Bn, Qn, NH, L, P, 2)
    attw = _softmax(aw.reshape(Bn, Qn, NH, L * P), -1)
    attw = attw.reshape(Bn, Qn, NH, L, P).transpose(0, 2, 1, 3, 4)
    norm = np.array([[w, h] for (h, w) in SHAPES], np.float32)
    loc = ref[:, :, None, :, None, :] + off / norm[None, None, None, :, None, :]
    loc = loc.transpose(0, 2, 1, 3, 4, 5)
    out = _sample(value, loc, attw)                      # (B,NH,Qn,DH)
    out = out.transpose(0, 2, 1, 3).reshape(Bn * Qn, D)
    if dev_out_proj and Qn == S:
        return (_dev_mm(out, p["out_w"]) + p["out_b"]).reshape(Bn, Qn, D)
    return (out @ p["out_w"] + p["out_b"]).reshape(Bn, Qn, D)


def _project_value(src, p):
    vf = _dev_mm(src.reshape(B * S, D), p["val_w"]) + p["val_b"]
    return vf.reshape(B, S, NH, DH).transpose(0, 2, 1, 3)


def kernel(src_flatten, pos_embed, query_embed, ref_w, ref_b,
           enc_params, dec_params):
    src_flatten = np.asarray(src_flatten, np.float32)
    pos_embed = np.asarray(pos_embed, np.float32)
    query_embed = np.asarray(query_embed, np.float32)
    ref_w = np.asarray(ref_w, np.float32)
    ref_b = np.asarray(ref_b, np.float32)
    enc_params = {k: np.asarray(v, np.float32) for k, v in enc_params.items()}
    dec_params = {k: np.asarray(v, np.float32) for k, v in dec_params.items()}

    ref_enc = _enc_reference_points()

    # ------------------------------------------------ encoder
    norm = np.array([[w, h] for (h, w) in SHAPES], np.float32)
    x = src_flatten
    for li in range(NLE):
        p = {k: v[li] for k, v in enc_params.items()}
        off, aw, vf = _dev_proj_bundle(
            x.reshape(B * S, D), pos_embed.reshape(B * S, D), p)
        value = (vf + p["val_b"]).reshape(B, S, NH, DH).transpose(0, 2, 1, 3)
        off = (off + p["off_b"]).reshape(B, S, NH, L, P, 2)
        aw = aw + p["attw_b"]
        attw = _softmax(aw.reshape(B, S, NH, L * P), -1)
        attw = attw.reshape(B, S, NH, L, P).transpose(0, 2, 1, 3, 4)
        loc = (ref_enc[:, :, None, :, None, :]
               + off / norm[None, None, None, :, None, :])
        loc = loc.transpose(0, 2, 1, 3, 4, 5)
        sam = _sample(value, loc, attw)
        sam = sam.transpose(0, 2, 1, 3).reshape(B * S, D)
        attn = (_dev_mm(sam, p["out_w"]) + p["out_b"]).reshape(B, S, D)
        x = _layer_norm(x + attn, p["ln1_g"], p["ln1_b"])
        ff = _dev_ffn(x.reshape(B * S, D), p) + p["f2_b"]
        x = _layer_norm(x + ff.reshape(B, S, D), p["ln2_g"], p["ln2_b"])
    memory = x

    # ------------------------------------------------ decoder
    qpos = np.broadcast_to(query_embed[None, :, :D], (B, Q, D))
    tgt = np.broadcast_to(query_embed[None, :, D:], (B, Q, D)).copy()
    ref = 1.0 / (1.0 + np.exp(-(query_embed[:, :D] @ ref_w + ref_b)))
    ref_in = np.broadcast_to(ref[None, :, None, :], (B, Q, L, 2))
    scale = 1.0 / np.sqrt(DH)

    x = tgt
    for li in range(NLD):
        p = {k: v[li] for k, v in dec_params.items()}
        q = x + qpos
        qh = (q @ p["qkv_w"][:, :D] + p["qkv_b"][:D]).reshape(B, Q, NH, DH)
        kh = (q @ p["qkv_w"][:, D:2 * D]
              + p["qkv_b"][D:2 * D]).reshape(B, Q, NH, DH)
        vh = (x @ p["qkv_w"][:, 2 * D:]
              + p["qkv_b"][2 * D:]).reshape(B, Q, NH, DH)
        att = _softmax(
            np.einsum("bqhd,bkhd->bhqk", qh, kh) * scale, -1)
        sa = (np.einsum("bhqk,bkhd->bqhd", att, vh).reshape(B, Q, D)
              @ p["oa_w"] + p["oa_b"])
        x = _layer_norm(x + sa, p["ln2_g"], p["ln2_b"])
        value = _project_value(memory, p)
        attn = _msda(x + qpos, ref_in, value, p)
        x = _layer_norm(x + attn, p["ln1_g"], p["ln1_b"])
        h = np.maximum(x @ p["f1_w"] + p["f1_b"], 0.0)
        x = _layer_norm(x + h @ p["f2_w"] + p["f2_b"],
                        p["ln3_g"], p["ln3_b"])
    return x.astype(np.float32)
